# revision 1
# baseline (speedup 1.0000x reference)
"""GPT-2 transformer block on 8 trn2 NeuronCores (Bass/Tile).

Sharding: token-split. Core c = 4*b + j handles batch b, output tokens
[512j, 512j+512). LN1 + KV projections are computed for the batch's full
sequence on every core of that batch group (needed for causal attention);
Q / attention / o-proj / MLP / output run only on the core's own 512 tokens.
No collectives; the host concatenates the 8 output slices.

All heavy matmuls run in float32r (TF32-like) with fp32 PSUM accumulation;
the residual stream stays fp32. Softmax uses the scoresT [sk, sq] layout:
exp (no max subtraction -- scores are bounded ~4 for this distribution),
post-exp causal mask multiply (mask is a per-core input), denominator via a
ones-column matmul, normalization via a K=1 broadcast matmul.
"""
import math
import os
import sys
import types

sys.path.insert(0, '/opt/trn_rl_repo')

import numpy as np


def _install_ntff_shim():
    """concourse's trace path imports antenv.axon_hooks, which this image
    lacks; give it a functional stand-in so trace=True doesn't crash."""
    try:
        import antenv.axon_hooks  # noqa: F401
        return
    except ImportError:
        pass
    try:
        import antenv
    except ImportError:
        return
    mod = types.ModuleType("antenv.axon_hooks")
    mod._hook = None

    def set_axon_ntff_profile_hook(h):
        mod._hook = h

    def get_axon_ntff_profile_hook():
        return mod._hook

    mod.set_axon_ntff_profile_hook = set_axon_ntff_profile_hook
    mod.get_axon_ntff_profile_hook = get_axon_ntff_profile_hook
    sys.modules["antenv.axon_hooks"] = mod
    antenv.axon_hooks = mod
    try:
        from trn_agent_boot.trn_boot import _ntff_profile_via_ctypes
        hook = _ntff_profile_via_ctypes('/opt/axon/libaxon_pjrt.so')
        if hook is not None:
            set_axon_ntff_profile_hook(hook)
    except Exception:
        pass


_install_ntff_shim()

import concourse.bass as bass
import concourse.tile as tile
from concourse import mybir, bass_utils
from concourse.masks import make_identity

P = 128
B, S, E = 2, 2048, 2048
H, D, KH, G = 16, 128, 4, 4
F = 8192
OWN = 512                 # tokens owned per core
NE = E // P               # 16
NSK = S // P              # 16
NF = F // P               # 64
NMS = OWN // P            # 4
f32 = mybir.dt.float32
f32r = mybir.dt.float32r
EXP_SCALE = 1.0 / math.sqrt(D)


def split_waits(nc, maxw=1):
    """This walrus build supports at most one sync-wait per instruction;
    hoist excess waits onto same-engine NoOps placed before the owner."""
    n = 0
    for fn in nc.m.functions:
        for blk in fn.blocks:
            new_insts = []
            for inst in blk.instructions:
                si = inst.sync_info
                if si is not None and si.on_wait and len(si.on_wait) > maxw:
                    waits = list(si.on_wait)
                    excess, keep = waits[:-maxw], waits[-maxw:]
                    for ci, w in enumerate(excess):
                        new_insts.append(mybir.InstNoOp(
                            name=f"{inst.name}-ws{ci}", engine=inst.engine,
                            sync_info=mybir.SyncInfo(on_wait=[w], on_update=[])))
                        n += 1
                    inst.sync_info = mybir.SyncInfo(
                        on_wait=keep, on_update=list(si.on_update or []))
                new_insts.append(inst)
            blk.instructions = new_insts
    return n


def _layernorm_tile(nc, pool, x_tile, g_b, b_b, eps_t, out_tile):
    """LayerNorm along the free dim (E) of x_tile [P, E] -> out_tile fp32."""
    stats = pool.tile([P, E // 512, 6], f32, tag="ln_stats")
    for i in range(E // 512):
        nc.vector.bn_stats(out=stats[:, i, :], in_=x_tile[:, i * 512:(i + 1) * 512])
    mv = pool.tile([P, 2], f32, tag="ln_mv")
    nc.vector.bn_aggr(out=mv, in_=stats)
    rstd = pool.tile([P, 1], f32, tag="ln_rstd")
    nc.scalar.activation(out=rstd, in_=mv[:, 1:2],
                         func=mybir.ActivationFunctionType.Sqrt, bias=eps_t)
    nc.vector.reciprocal(out=rstd, in_=rstd)
    nc.vector.tensor_scalar(out=out_tile, in0=x_tile, scalar1=mv[:, 0:1],
                            scalar2=rstd, op0=mybir.AluOpType.subtract,
                            op1=mybir.AluOpType.mult)
    nc.vector.tensor_mul(out_tile, out_tile, g_b)
    nc.vector.tensor_add(out_tile, out_tile, b_b)


def _ln_transpose_strips(nc, pool, tp_psum_pool, x_src, tok_tiles, g_b, b_b,
                         eps_t, ident, strips, xtag):
    """Stream token tiles of x_src, LayerNorm them, transpose into the given
    e-major strips: strips[e][:, 128*t : 128*t+128] = LN(x)[t-tile, e-tile]^T."""
    for t in range(tok_tiles):
        x_t = pool.tile([P, E], f32, tag=f"{xtag}_x")
        nc.sync.dma_start(out=x_t, in_=x_src[t * P:(t + 1) * P, :])
        x1_t = pool.tile([P, E], f32, tag=f"{xtag}_x1")
        _layernorm_tile(nc, pool, x_t, g_b, b_b, eps_t, x1_t)
        for e in range(NE):
            tp = tp_psum_pool.tile([P, P], f32, tag="tp", bufs=2)
            nc.tensor.transpose(tp, x1_t[:, e * P:(e + 1) * P], ident)
            nc.scalar.copy(strips[e][:, t * P:(t + 1) * P], tp)


def build():
    nc = bass.Bass("TRN2", target_bir_lowering=False, debug=False, num_devices=8)

    xkv = nc.dram_tensor("xkv", [S, E], f32, kind="ExternalInput").ap()
    xow = nc.dram_tensor("xow", [OWN, E], f32, kind="ExternalInput").ap()
    maskd = nc.dram_tensor("mask", [S, OWN], f32, kind="ExternalInput").ap()
    wq_s = nc.dram_tensor("wq_s", [H, E, P], f32r, kind="ExternalInput").ap()
    wk_s = nc.dram_tensor("wk_s", [KH, E, P], f32r, kind="ExternalInput").ap()
    wv_s = nc.dram_tensor("wv_s", [KH, E, P], f32r, kind="ExternalInput").ap()
    wo_t = nc.dram_tensor("wo_t", [H, 4, P, 512], f32r, kind="ExternalInput").ap()
    wu_s = nc.dram_tensor("wu_s", [NF, E, P], f32r, kind="ExternalInput").ap()
    wd_t = nc.dram_tensor("wd_t", [NF, 4, P, 512], f32r, kind="ExternalInput").ap()
    bq = nc.dram_tensor("bq", [E], f32, kind="ExternalInput").ap()
    bk = nc.dram_tensor("bk", [KH * D], f32, kind="ExternalInput").ap()
    bv = nc.dram_tensor("bv", [KH * D], f32, kind="ExternalInput").ap()
    bo = nc.dram_tensor("bo", [E], f32, kind="ExternalInput").ap()
    bu = nc.dram_tensor("bu", [F], f32, kind="ExternalInput").ap()
    bd = nc.dram_tensor("bd", [E], f32, kind="ExternalInput").ap()
    g1 = nc.dram_tensor("g1", [E], f32, kind="ExternalInput").ap()
    b1 = nc.dram_tensor("b1", [E], f32, kind="ExternalInput").ap()
    g2 = nc.dram_tensor("g2", [E], f32, kind="ExternalInput").ap()
    b2 = nc.dram_tensor("b2", [E], f32, kind="ExternalInput").ap()
    out = nc.dram_tensor("out", [OWN, E], f32, kind="ExternalOutput").ap()

    with tile.TileContext(nc) as tc:
        _build_body(nc, tc, locals())
    return nc


def _build_body(nc, tc, t_):
    xkv, xow, maskd = t_["xkv"], t_["xow"], t_["maskd"]
    wq_s, wk_s, wv_s, wo_t, wu_s, wd_t = (t_[k] for k in
                                          ("wq_s", "wk_s", "wv_s", "wo_t", "wu_s", "wd_t"))
    bq, bk, bv, bo, bu, bd = (t_[k] for k in ("bq", "bk", "bv", "bo", "bu", "bd"))
    g1, b1, g2, b2, out = (t_[k] for k in ("g1", "b1", "g2", "b2", "out"))
    Ident = mybir.ActivationFunctionType.Identity
    Exp = mybir.ActivationFunctionType.Exp
    Gelu = mybir.ActivationFunctionType.Gelu
    mult = mybir.AluOpType.mult
    add = mybir.AluOpType.add

    with (
        tc.tile_pool(name="persist", bufs=1) as persist,
        tc.tile_pool(name="dram", bufs=1, space="DRAM") as dram,
    ):
        ident = persist.tile([P, P], f32)
        make_identity(nc, ident)
        eps_t = persist.tile([P, 1], f32)
        nc.vector.memset(eps_t, 1e-5)
        ones_col = persist.tile([P, 1], f32)   # lhsT for denominator (K=P, M=1)
        nc.vector.memset(ones_col, 1.0)
        ones_row = persist.tile([1, P], f32)   # lhsT for broadcast (K=1, M=P)
        nc.vector.memset(ones_row, 1.0)
        bq_sb = persist.tile([P, H], f32)
        nc.sync.dma_start(out=bq_sb, in_=bq.rearrange("(t p) -> p t", p=P))
        bk_sb = persist.tile([P, KH], f32)
        nc.sync.dma_start(out=bk_sb, in_=bk.rearrange("(t p) -> p t", p=P))
        bv_sb = persist.tile([P, KH], f32)
        nc.sync.dma_start(out=bv_sb, in_=bv.rearrange("(t p) -> p t", p=P))
        xmid_dram = dram.tile([OWN, E], f32)

        with tc.tile_pool(name="qkv_keep", bufs=1) as qkv_keep:
            qT = [qkv_keep.tile([P, OWN], f32r, tag=f"qT{i}", name=f"qT{i}") for i in range(H)]
            kT = [qkv_keep.tile([P, S], f32r, tag=f"kT{i}", name=f"kT{i}") for i in range(KH)]
            vtok = [qkv_keep.tile([P, KH * D], f32r, tag=f"vtok{i}", name=f"vtok{i}") for i in range(NSK)]

            # ---------------- Phase A: Q projections for own tokens ----------
            with (
                tc.tile_pool(name="pA", bufs=1) as pA,
                tc.tile_pool(name="psA", bufs=1, space="PSUM") as psA,
            ):
                g1_b = pA.tile([P, E], f32, tag="g1b")
                nc.sync.dma_start(out=g1_b, in_=g1.unsqueeze(0).to_broadcast((P, E)))
                b1_b = pA.tile([P, E], f32, tag="b1b")
                nc.sync.dma_start(out=b1_b, in_=b1.unsqueeze(0).to_broadcast((P, E)))
                x1own = [pA.tile([P, OWN], f32r, tag=f"x1own{e}", name=f"x1own{e}") for e in range(NE)]
                _ln_transpose_strips(nc, pA, psA, xow, NMS, g1_b, b1_b,
                                     eps_t, ident, x1own, "A")
                for mg in range(H // 2):
                    for mi in range(2):
                        m = mg * 2 + mi
                        wstrip = pA.tile([P, NE, P], f32r, tag=f"wq{mi}")
                        nc.sync.dma_start(
                            out=wstrip,
                            in_=wq_s[m].rearrange("(t p) m -> p t m", p=P))
                        psq = psA.tile([P, OWN], f32, tag=f"psq{mi}", bufs=2)
                        for e in range(NE):
                            nc.tensor.matmul(psq, wstrip[:, e, :], x1own[e],
                                             start=(e == 0), stop=(e == NE - 1))
                        nc.scalar.activation(out=qT[m], in_=psq, func=Ident,
                                             bias=bq_sb[:, m:m + 1])

            # ---------------- Phase B: K/V for the full sequence -------------
            with (
                tc.tile_pool(name="pB", bufs=1) as pB,
                tc.tile_pool(name="psB", bufs=1, space="PSUM") as psB,
            ):
                g1_b = pB.tile([P, E], f32, tag="g1b")
                nc.sync.dma_start(out=g1_b, in_=g1.unsqueeze(0).to_broadcast((P, E)))
                b1_b = pB.tile([P, E], f32, tag="b1b")
                nc.sync.dma_start(out=b1_b, in_=b1.unsqueeze(0).to_broadcast((P, E)))
                for c in range(S // OWN):
                    x1c = [pB.tile([P, OWN], f32r, tag=f"x1c{e}", name=f"x1c{e}") for e in range(NE)]
                    _ln_transpose_strips(nc, pB, psB,
                                         xkv[c * OWN:(c + 1) * OWN, :], NMS,
                                         g1_b, b1_b, eps_t, ident, x1c, "B")
                    for kv_or_v in range(2):
                        w_src, b_sb = ((wk_s, bk_sb), (wv_s, bv_sb))[kv_or_v]
                        for mg in range(2):
                            strips = []
                            for mi in range(2):
                                m = mg * 2 + mi
                                wstrip = pB.tile([P, NE, P], f32r, tag=f"wkv{mi}")
                                nc.sync.dma_start(
                                    out=wstrip,
                                    in_=w_src[m].rearrange("(t p) m -> p t m", p=P))
                                strips.append(wstrip)
                            for mi in range(2):
                                m = mg * 2 + mi
                                pskv = psB.tile([P, OWN], f32, tag=f"pskv{mi}", bufs=2)
                                for e in range(NE):
                                    nc.tensor.matmul(pskv, strips[mi][:, e, :], x1c[e],
                                                     start=(e == 0), stop=(e == NE - 1))
                                if kv_or_v == 0:
                                    nc.scalar.activation(
                                        out=kT[m][:, c * OWN:(c + 1) * OWN],
                                        in_=pskv, func=Ident, bias=b_sb[:, m:m + 1])
                                else:
                                    vf = pB.tile([P, OWN], f32, tag=f"vf{mi}")
                                    nc.scalar.activation(out=vf, in_=pskv, func=Ident,
                                                         bias=b_sb[:, m:m + 1])
                                    for t in range(NMS):
                                        tp = psB.tile([P, P], f32, tag="vtp", bufs=2)
                                        nc.tensor.transpose(
                                            tp, vf[:, t * P:(t + 1) * P], ident)
                                        nc.scalar.copy(
                                            vtok[c * NMS + t][:, m * P:(m + 1) * P], tp)

            # ---------------- Phase C+D: attention, o-proj ------------------
            with tc.tile_pool(name="oT_keep", bufs=1) as oT_keep:
                oT = [oT_keep.tile([P, OWN], f32r, tag=f"oT{i}", name=f"oT{i}") for i in range(H)]
                with (
                    tc.tile_pool(name="pC", bufs=1) as pC,
                    tc.tile_pool(name="psC", bufs=1, space="PSUM") as psC,
                ):
                    masks = [pC.tile([P, OWN], f32, tag=f"mask{i}", name=f"mask{i}") for i in range(NSK)]
                    for i in range(NSK):
                        nc.sync.dma_start(out=masks[i], in_=maskd[i * P:(i + 1) * P, :])
                    for h in range(H):
                        kv = h // G
                        ps_o = psC.tile([P, OWN], f32, tag="ps_o", bufs=1)
                        ps_den = psC.tile([1, OWN], f32, tag="ps_den", bufs=1)
                        for sk in range(NSK):
                            ps_s = psC.tile([P, OWN], f32, tag="ps_s", bufs=2)
                            nc.tensor.matmul(ps_s, kT[kv][:, sk * P:(sk + 1) * P],
                                             qT[h], start=True, stop=True)
                            ex = pC.tile([P, OWN], f32, tag="ex", bufs=2)
                            nc.scalar.activation(out=ex, in_=ps_s, func=Exp,
                                                 scale=EXP_SCALE)
                            exr = pC.tile([P, OWN], f32r, tag="exr", bufs=3)
                            nc.vector.tensor_tensor(out=exr, in0=ex, in1=masks[sk],
                                                    op=mult)
                            nc.tensor.matmul(ps_o, vtok[sk][:, kv * P:(kv + 1) * P],
                                             exr, start=(sk == 0), stop=(sk == NSK - 1))
                            nc.tensor.matmul(ps_den, ones_col.bitcast(f32r), exr,
                                             start=(sk == 0), stop=(sk == NSK - 1))
                        rden = pC.tile([1, OWN], f32r, tag="rden", bufs=2)
                        with nc.allow_low_precision(reason="softmax denominator"):
                            nc.vector.reciprocal(out=rden, in_=ps_den)
                        ps_bc = psC.tile([P, OWN], f32, tag="ps_bc", bufs=1)
                        nc.tensor.matmul(ps_bc, ones_row.bitcast(f32r), rden,
                                         start=True, stop=True)
                        bc = pC.tile([P, OWN], f32, tag="bc", bufs=2)
                        nc.vector.tensor_copy(bc, ps_bc)
                        nc.vector.tensor_tensor(out=oT[h], in0=ps_o, in1=bc, op=mult)

                with (
                    tc.tile_pool(name="pD", bufs=1) as pD,
                    tc.tile_pool(name="psD", bufs=1, space="PSUM") as psD,
                ):
                    bo_b = pD.tile([P, E], f32, tag="bo_b")
                    nc.sync.dma_start(out=bo_b, in_=bo.unsqueeze(0).to_broadcast((P, E)))
                    xow_sb = [pD.tile([P, E], f32, tag=f"xow{t}", name=f"xow{t}") for t in range(NMS)]
                    for t in range(NMS):
                        nc.sync.dma_start(out=xow_sb[t], in_=xow[t * P:(t + 1) * P, :])
                    for ec in range(4):
                        pso1 = [psD.tile([P, 512], f32, tag=f"pso1_{ms}", bufs=1, name=f"pso1_{ms}")
                                for ms in range(NMS)]
                        for k in range(H):
                            wtile = pD.tile([P, 512], f32r, tag="wo", bufs=3)
                            nc.sync.dma_start(out=wtile, in_=wo_t[k, ec])
                            for ms in range(NMS):
                                nc.tensor.matmul(pso1[ms], oT[k][:, ms * P:(ms + 1) * P],
                                                 wtile, start=(k == 0), stop=(k == H - 1))
                        for ms in range(NMS):
                            xm = pD.tile([P, 512], f32, tag="xm", bufs=3)
                            nc.vector.tensor_tensor(
                                out=xm, in0=pso1[ms],
                                in1=xow_sb[ms][:, ec * 512:(ec + 1) * 512], op=add)
                            nc.vector.tensor_tensor(
                                out=xm, in0=xm,
                                in1=bo_b[:, ec * 512:(ec + 1) * 512], op=add)
                            nc.sync.dma_start(
                                out=xmid_dram[ms * P:(ms + 1) * P, ec * 512:(ec + 1) * 512],
                                in_=xm)

        # ---------------- Phase E-G: LN2, MLP ---------------------------
        with tc.tile_pool(name="mlp_keep", bufs=1) as mlp_keep:
            x2T = [mlp_keep.tile([P, OWN], f32r, tag=f"x2T{e}", name=f"x2T{e}") for e in range(NE)]
            hT = [mlp_keep.tile([P, OWN], f32r, tag=f"hT{i}", name=f"hT{i}") for i in range(NF // 2)]
            outp = [mlp_keep.tile([P, E], f32, tag=f"outp{t}", name=f"outp{t}") for t in range(NMS)]

            with (
                tc.tile_pool(name="pE", bufs=1) as pE,
                tc.tile_pool(name="psE", bufs=1, space="PSUM") as psE,
            ):
                g2_b = pE.tile([P, E], f32, tag="g2b")
                nc.sync.dma_start(out=g2_b, in_=g2.unsqueeze(0).to_broadcast((P, E)))
                b2_b = pE.tile([P, E], f32, tag="b2b")
                nc.sync.dma_start(out=b2_b, in_=b2.unsqueeze(0).to_broadcast((P, E)))
                _ln_transpose_strips(nc, pE, psE, xmid_dram,
                                     NMS, g2_b, b2_b, eps_t, ident, x2T, "E")

            with (
                tc.tile_pool(name="pF", bufs=1) as pF,
                tc.tile_pool(name="psF", bufs=1, space="PSUM") as psF,
            ):
                bu_sb = pF.tile([P, NF], f32)
                nc.sync.dma_start(out=bu_sb, in_=bu.rearrange("(t p) -> p t", p=P))
                bd_b = pF.tile([P, E], f32)
                nc.sync.dma_start(out=bd_b, in_=bd.unsqueeze(0).to_broadcast((P, E)))
                for fh in range(2):
                    # ---- up half ----
                    for fi in range(NF // 2):
                        f = fh * (NF // 2) + fi
                        wstrip = pF.tile([P, NE, P], f32r, tag="wu", bufs=2)
                        nc.sync.dma_start(
                            out=wstrip, in_=wu_s[f].rearrange("(t p) m -> p t m", p=P))
                        psh = psF.tile([P, OWN], f32, tag="psh", bufs=2)
                        for e in range(NE):
                            nc.tensor.matmul(psh, wstrip[:, e, :], x2T[e],
                                             start=(e == 0), stop=(e == NE - 1))
                        nc.scalar.activation(out=hT[fi], in_=psh, func=Gelu,
                                             bias=bu_sb[:, f:f + 1])
                    # ---- down half ----
                    for ec in range(4):
                        psd = [psF.tile([P, 512], f32, tag=f"psd{ms}", bufs=1, name=f"psd{ms}")
                               for ms in range(NMS)]
                        for fi in range(NF // 2):
                            f = fh * (NF // 2) + fi
                            wtile = pF.tile([P, 512], f32r, tag="wd", bufs=3)
                            nc.sync.dma_start(out=wtile, in_=wd_t[f, ec])
                            for ms in range(NMS):
                                nc.tensor.matmul(psd[ms], hT[fi][:, ms * P:(ms + 1) * P],
                                                 wtile, start=(fi == 0),
                                                 stop=(fi == NF // 2 - 1))
                        for ms in range(NMS):
                            if fh == 0:
                                nc.vector.tensor_copy(
                                    outp[ms][:, ec * 512:(ec + 1) * 512], psd[ms])
                            else:
                                nc.vector.tensor_tensor(
                                    out=outp[ms][:, ec * 512:(ec + 1) * 512],
                                    in0=psd[ms],
                                    in1=outp[ms][:, ec * 512:(ec + 1) * 512], op=add)

                # ---- final: out = outp + xmid + bd ----
                for ms in range(NMS):
                    xm = pF.tile([P, E], f32, tag="xm2", bufs=2)
                    nc.sync.dma_start(out=xm, in_=xmid_dram[ms * P:(ms + 1) * P, :])
                    nc.vector.tensor_tensor(out=outp[ms], in0=outp[ms], in1=xm, op=add)
                    nc.vector.tensor_tensor(out=outp[ms], in0=outp[ms], in1=bd_b, op=add)
                    nc.sync.dma_start(out=out[ms * P:(ms + 1) * P, :], in_=outp[ms])


_NC_CACHE = None
LAST_RESULTS = None


def _get_nc():
    global _NC_CACHE
    if _NC_CACHE is None:
        nc = build()
        split_waits(nc)
        _NC_CACHE = nc
    return _NC_CACHE


def _prep_shared(wq, wk, wv, wo, wu, wd):
    def strips(w, n):  # [E, n*128] -> [n, E, 128]
        return np.ascontiguousarray(w.reshape(w.shape[0], n, P).transpose(1, 0, 2))

    def tiles(w, nr):  # [nr*128, E] -> [nr, 4, 128, 512]
        return np.ascontiguousarray(
            w.reshape(nr, P, 4, 512).transpose(0, 2, 1, 3))

    return {
        "wq_s": strips(np.asarray(wq, np.float32), H),
        "wk_s": strips(np.asarray(wk, np.float32), KH),
        "wv_s": strips(np.asarray(wv, np.float32), KH),
        "wo_t": tiles(np.asarray(wo, np.float32), H),
        "wu_s": strips(np.asarray(wu, np.float32), NF),
        "wd_t": tiles(np.asarray(wd, np.float32), NF),
    }


def kernel(x, ln1_g, ln1_b, wq, bq, wk, bk, wv, bv, wo, bo, ln2_g, ln2_b,
           wu, bu, wd, bd):
    x = np.asarray(x, np.float32)
    shared = _prep_shared(wq, wk, wv, wo, wu, wd)
    shared.update({
        "bq": np.asarray(bq, np.float32), "bk": np.asarray(bk, np.float32),
        "bv": np.asarray(bv, np.float32), "bo": np.asarray(bo, np.float32),
        "bu": np.asarray(bu, np.float32), "bd": np.asarray(bd, np.float32),
        "g1": np.asarray(ln1_g, np.float32), "b1": np.asarray(ln1_b, np.float32),
        "g2": np.asarray(ln2_g, np.float32), "b2": np.asarray(ln2_b, np.float32),
    })
    sk_idx = np.arange(S)[:, None]
    in_maps = []
    for core in range(8):
        b, j = divmod(core, 4)
        m = dict(shared)
        m["xkv"] = np.ascontiguousarray(x[b])
        m["xow"] = np.ascontiguousarray(x[b, OWN * j:OWN * (j + 1)])
        sq_idx = OWN * j + np.arange(OWN)[None, :]
        m["mask"] = (sk_idx <= sq_idx).astype(np.float32)
        in_maps.append(m)

    nc = _get_nc()
    trace = bool(os.environ.get("KERNEL_TRACE"))
    res = bass_utils.run_bass_kernel_spmd(
        nc, in_maps, core_ids=list(range(8)), trace=trace)
    global LAST_RESULTS
    LAST_RESULTS = res
    out = np.empty((B, S, E), np.float32)
    for core in range(8):
        b, j = divmod(core, 4)
        out[b, OWN * j:OWN * (j + 1)] = res.results[core]["out"]
    return out



# revision 12
# speedup vs baseline: 1.4763x; 1.4763x over previous
"""GPT-2 transformer block on 8 trn2 NeuronCores (Bass/Tile).

Sharding: balanced-causal token split. Core c = 4*b + j handles batch b and
owns two 256-token chunks: abs chunks j and 7-j (512 tokens total), so every
core has the same causal attention workload. Each core LayerNorms + QKV-
projects only its own tokens; K/V are exchanged within the 4-core batch group
via an HBM AllGather, with each contribution landing at absolute positions
keyed by the compile-time replica slot. Attention uses uniform loop bounds
(sk tiles 0..7 for the low q-half, 0..15 for the high q-half) with per-core
causal masks supplied as input data; per-core structural skip is impossible
under SPMD. MLP is token-parallel (no collectives).

All matmul operands are bf16 (fp32 PSUM accumulation); the residual stream is
fp32 and never leaves SBUF between attention and MLP. Weight tensors are
pre-transposed on the host so every weight DMA is a contiguous [128, 2048]
block (4KB per partition line). Softmax: exp on the scalar engine (scores
bounded, no max subtraction), causal mask multiply + denominator accumulation
on the vector engine, single ones-column matmul per head for the partition
reduction.
"""
import math
import os
import sys
import types

sys.path.insert(0, '/opt/trn_rl_repo')

import numpy as np
from ml_dtypes import bfloat16 as np_bf16


def _install_ntff_shim():
    """concourse's trace path imports antenv.axon_hooks, which this image
    lacks; give it a functional stand-in so trace=True doesn't crash."""
    try:
        import antenv.axon_hooks  # noqa: F401
        return
    except ImportError:
        pass
    try:
        import antenv
    except ImportError:
        return
    mod = types.ModuleType("antenv.axon_hooks")
    mod._hook = None

    def set_axon_ntff_profile_hook(h):
        mod._hook = h

    def get_axon_ntff_profile_hook():
        return mod._hook

    mod.set_axon_ntff_profile_hook = set_axon_ntff_profile_hook
    mod.get_axon_ntff_profile_hook = get_axon_ntff_profile_hook
    sys.modules["antenv.axon_hooks"] = mod
    antenv.axon_hooks = mod
    try:
        from trn_agent_boot.trn_boot import _ntff_profile_via_ctypes
        hook = _ntff_profile_via_ctypes('/opt/axon/libaxon_pjrt.so')
        if hook is not None:
            set_axon_ntff_profile_hook(hook)
    except Exception:
        pass


_install_ntff_shim()

import concourse.bass as bass
import concourse.tile as tile
from concourse import mybir, bass_utils
from concourse.masks import make_identity

P = 128
B, S, E = 2, 2048, 2048
H, D, KH, G = 16, 128, 4, 4
F = 8192
OWN = 512                 # tokens owned per core (two 256 chunks)
CH = 256                  # ownership chunk
NE = E // P               # 16
NSK = S // P              # 16
NF = F // P               # 64
NMS = OWN // P            # 4
f32 = mybir.dt.float32
f32r = mybir.dt.float32r
bf16 = mybir.dt.bfloat16
EXP_SCALE = 1.0 / math.sqrt(D)


def split_waits(nc, maxw=1):
    """This walrus build supports at most one sync-wait per instruction;
    hoist excess waits onto same-engine NoOps placed before the owner."""
    n = 0
    for fn in nc.m.functions:
        for blk in fn.blocks:
            new_insts = []
            for inst in blk.instructions:
                si = inst.sync_info
                if si is not None and si.on_wait and len(si.on_wait) > maxw:
                    waits = list(si.on_wait)
                    excess, keep = waits[:-maxw], waits[-maxw:]
                    for ci, w in enumerate(excess):
                        new_insts.append(mybir.InstNoOp(
                            name=f"{inst.name}-ws{ci}", engine=inst.engine,
                            sync_info=mybir.SyncInfo(on_wait=[w], on_update=[])))
                        n += 1
                    inst.sync_info = mybir.SyncInfo(
                        on_wait=keep, on_update=list(si.on_update or []))
                new_insts.append(inst)
            blk.instructions = new_insts
    return n


def own_abs_tiles(p):
    """Absolute 128-token sk-tile indices of core-slot p's four token tiles."""
    return [2 * p, 2 * p + 1, 14 - 2 * p, 15 - 2 * p]


def _ln_normalize(nc, pool, x_tile, eps_t, out_tile, tag):
    """out = (x - mean) * rsqrt(var + eps) along the free dim (E), fp32."""
    stats = pool.tile([P, E // 512, 6], f32, tag=f"{tag}_stats")
    for i in range(E // 512):
        nc.vector.bn_stats(out=stats[:, i, :], in_=x_tile[:, i * 512:(i + 1) * 512])
    mv = pool.tile([P, 2], f32, tag=f"{tag}_mv")
    nc.vector.bn_aggr(out=mv, in_=stats)
    rstd = pool.tile([P, 1], f32, tag=f"{tag}_rstd")
    nc.scalar.activation(out=rstd, in_=mv[:, 1:2],
                         func=mybir.ActivationFunctionType.Sqrt, bias=eps_t)
    nc.vector.reciprocal(out=rstd, in_=rstd)
    nc.vector.tensor_scalar(out=out_tile, in0=x_tile, scalar1=mv[:, 0:1],
                            scalar2=rstd, op0=mybir.AluOpType.subtract,
                            op1=mybir.AluOpType.mult)


def _ln_transpose_strips(nc, pool, psum_pool, x_tiles, g_col, b_col, eps_t,
                         ident, strips, tag):
    """LayerNorm token tiles and transpose into e-major bf16 strips with the
    affine (g, b) fused into the post-transpose copy (g/b become per-partition
    scalars after the transpose)."""
    Ident = mybir.ActivationFunctionType.Identity
    for t, x_t in enumerate(x_tiles):
        xn = pool.tile([P, E], f32, tag=f"{tag}_xn", bufs=2)
        _ln_normalize(nc, pool, x_t, eps_t, xn, tag)
        for e in range(NE):
            tp = psum_pool.tile([P, P], f32, tag=f"{tag}_tp", bufs=2)
            nc.tensor.transpose(tp, xn[:, e * P:(e + 1) * P], ident)
            nc.scalar.activation(out=strips[e][:, t * P:(t + 1) * P], in_=tp,
                                 func=Ident, scale=g_col[:, e:e + 1],
                                 bias=b_col[:, e:e + 1])


def build(debug=False):
    nc = bass.Bass("TRN2", target_bir_lowering=False, debug=debug,
                   num_devices=8)

    xow = nc.dram_tensor("xow", [OWN, E], f32, kind="ExternalInput").ap()
    mask_lo = nc.dram_tensor("mask_lo", [8, P, OWN], bf16,
                             kind="ExternalInput").ap()
    mask_hi = nc.dram_tensor("mask_hi", [8, P, CH], bf16,
                             kind="ExternalInput").ap()
    wq_b = nc.dram_tensor("wq_b", [H, P, NE * P], bf16, kind="ExternalInput").ap()
    wk_b = nc.dram_tensor("wk_b", [KH, P, NE * P], bf16, kind="ExternalInput").ap()
    wv_b = nc.dram_tensor("wv_b", [KH, P, NE * P], bf16, kind="ExternalInput").ap()
    wo_b = nc.dram_tensor("wo_b", [H, P, E], bf16, kind="ExternalInput").ap()
    wu_b = nc.dram_tensor("wu_b", [NF, P, NE * P], bf16, kind="ExternalInput").ap()
    wd_b = nc.dram_tensor("wd_b", [NF, P, E], bf16, kind="ExternalInput").ap()
    bq_c = nc.dram_tensor("bq_c", [P, H], f32, kind="ExternalInput").ap()
    bk_c = nc.dram_tensor("bk_c", [P, KH], f32, kind="ExternalInput").ap()
    bv_c = nc.dram_tensor("bv_c", [P, KH], f32, kind="ExternalInput").ap()
    bu_c = nc.dram_tensor("bu_c", [P, NF], f32, kind="ExternalInput").ap()
    g1_c = nc.dram_tensor("g1_c", [P, NE], f32, kind="ExternalInput").ap()
    b1_c = nc.dram_tensor("b1_c", [P, NE], f32, kind="ExternalInput").ap()
    g2_c = nc.dram_tensor("g2_c", [P, NE], f32, kind="ExternalInput").ap()
    b2_c = nc.dram_tensor("b2_c", [P, NE], f32, kind="ExternalInput").ap()
    bo_r = nc.dram_tensor("bo_r", [E], f32, kind="ExternalInput").ap()
    bd_r = nc.dram_tensor("bd_r", [E], f32, kind="ExternalInput").ap()
    out = nc.dram_tensor("out", [OWN, E], f32, kind="ExternalOutput").ap()

    with tile.TileContext(nc) as tc:
        _build_body(nc, tc, locals())
    return nc


def _build_body(nc, tc, t_):
    xow, mask_lo, mask_hi = t_["xow"], t_["mask_lo"], t_["mask_hi"]
    wq_b, wk_b, wv_b, wo_b, wu_b, wd_b = (t_[k] for k in
                                          ("wq_b", "wk_b", "wv_b", "wo_b",
                                           "wu_b", "wd_b"))
    bq_c, bk_c, bv_c, bu_c = (t_[k] for k in ("bq_c", "bk_c", "bv_c", "bu_c"))
    g1_c, b1_c, g2_c, b2_c = (t_[k] for k in ("g1_c", "b1_c", "g2_c", "b2_c"))
    bo_r, bd_r, out = t_["bo_r"], t_["bd_r"], t_["out"]
    Ident = mybir.ActivationFunctionType.Identity
    Exp = mybir.ActivationFunctionType.Exp
    Gelu = mybir.ActivationFunctionType.Gelu
    mult = mybir.AluOpType.mult
    add = mybir.AluOpType.add

    with (
        tc.tile_pool(name="persist", bufs=1) as persist,
        tc.tile_pool(name="dram", bufs=1, space="DRAM") as dram,
    ):
        ident = persist.tile([P, P], f32)
        make_identity(nc, ident)
        eps_t = persist.tile([P, 1], f32)
        nc.vector.memset(eps_t, 1e-5)
        ones_col = persist.tile([P, 1], bf16)   # lhsT for den (K=P, M=1)
        nc.vector.memset(ones_col, 1.0)
        ones_row = persist.tile([1, P], f32)    # lhsT for broadcast (K=1, M=P)
        nc.vector.memset(ones_row, 1.0)
        bq_sb = persist.tile([P, H], f32)
        nc.sync.dma_start(out=bq_sb, in_=bq_c)
        bk_sb = persist.tile([P, KH], f32)
        nc.sync.dma_start(out=bk_sb, in_=bk_c)
        bv_sb = persist.tile([P, KH], f32)
        nc.sync.dma_start(out=bv_sb, in_=bv_c)
        g1_sb = persist.tile([P, NE], f32)
        nc.sync.dma_start(out=g1_sb, in_=g1_c)
        b1_sb = persist.tile([P, NE], f32)
        nc.sync.dma_start(out=b1_sb, in_=b1_c)
        g2_sb = persist.tile([P, NE], f32)
        nc.sync.dma_start(out=g2_sb, in_=g2_c)
        b2_sb = persist.tile([P, NE], f32)
        nc.sync.dma_start(out=b2_sb, in_=b2_c)
        bo_b = persist.tile([P, E], f32)
        nc.sync.dma_start(out=bo_b, in_=bo_r.unsqueeze(0).to_broadcast((P, E)))
        bd_b = persist.tile([P, E], f32)
        nc.sync.dma_start(out=bd_b, in_=bd_r.unsqueeze(0).to_broadcast((P, E)))

        # DRAM bounce buffers for the K/V AllGather.
        # units 0..KH-1: kT (feat-major); units KH..KH+3: vtok (tok-major)
        kvx_in = dram.tile([KH + NMS, P, OWN], bf16)
        kvx_out = dram.tile([4, KH + NMS, P, OWN], bf16)

        # xt holds the residual stream: input x during attention, then is
        # overwritten in place with xmid = x + attn_out for the MLP phases.
        xt = [persist.tile([P, E], f32, name=f"xt{t}") for t in range(NMS)]

        with tc.tile_pool(name="attn_keep", bufs=1) as akeep:
            qT = [akeep.tile([P, OWN], bf16, tag=f"qT{i}", name=f"qT{i}")
                  for i in range(H)]
            kT = [akeep.tile([P, S], bf16, tag=f"kT{i}", name=f"kT{i}")
                  for i in range(KH)]
            vtok = [akeep.tile([P, KH * P], bf16, tag=f"vt{i}", name=f"vt{i}")
                    for i in range(NSK)]
            oT = [akeep.tile([P, OWN], bf16, tag=f"oT{i}", name=f"oT{i}")
                  for i in range(H)]

            # ---------------- Phase A: LN1 + QKV for own tokens ----------
            with (
                tc.tile_pool(name="pA", bufs=1) as pA,
                tc.tile_pool(name="psA", bufs=1, space="PSUM") as psA,
            ):
                for t in range(NMS):
                    nc.sync.dma_start(out=xt[t], in_=xow[t * P:(t + 1) * P, :])
                x1T = [pA.tile([P, OWN], bf16, tag=f"x1T{e}", name=f"x1T{e}")
                       for e in range(NE)]
                _ln_transpose_strips(nc, pA, psA, xt, g1_sb, b1_sb, eps_t,
                                     ident, x1T, "A")

                # K/V projections for own tokens -> exchange staging
                kstage = [pA.tile([P, OWN], bf16, tag=f"ks{kv}", name=f"ks{kv}")
                          for kv in range(KH)]
                vstage = [pA.tile([P, KH * P], bf16, tag=f"vs{t}", name=f"vs{t}")
                          for t in range(NMS)]
                for kv_or_v in range(2):
                    w_src, b_sb = ((wk_b, bk_sb), (wv_b, bv_sb))[kv_or_v]
                    for m in range(KH):
                        wstrip = pA.tile([P, NE * P], bf16, tag="wkv", bufs=3)
                        nc.sync.dma_start(out=wstrip, in_=w_src[m])
                        pskv = psA.tile([P, OWN], f32, tag="pskv", bufs=2)
                        for e in range(NE):
                            nc.tensor.matmul(pskv, wstrip[:, e * P:(e + 1) * P],
                                             x1T[e], start=(e == 0),
                                             stop=(e == NE - 1))
                        if kv_or_v == 0:
                            nc.scalar.activation(out=kstage[m], in_=pskv,
                                                 func=Ident,
                                                 bias=b_sb[:, m:m + 1])
                        else:
                            vf = pA.tile([P, OWN], f32, tag="vf", bufs=2)
                            nc.scalar.activation(out=vf, in_=pskv, func=Ident,
                                                 bias=b_sb[:, m:m + 1])
                            for t in range(NMS):
                                tp = psA.tile([P, P], f32, tag="vtp", bufs=2)
                                nc.tensor.transpose(
                                    tp, vf[:, t * P:(t + 1) * P], ident)
                                nc.scalar.copy(
                                    vstage[t][:, m * P:(m + 1) * P], tp)
                for kv in range(KH):
                    nc.gpsimd.dma_start(kvx_in[kv], kstage[kv])
                for t in range(NMS):
                    nc.gpsimd.dma_start(kvx_in[KH + t], vstage[t])
                nc.gpsimd.collective_compute(
                    "AllGather", mybir.AluOpType.bypass,
                    replica_groups=[[0, 1, 2, 3], [4, 5, 6, 7]],
                    ins=[kvx_in.opt()], outs=[kvx_out.opt()])

                # Q projections (overlap the collective)
                for m in range(H):
                    wstrip = pA.tile([P, NE * P], bf16, tag="wq", bufs=3)
                    nc.sync.dma_start(out=wstrip, in_=wq_b[m])
                    psq = psA.tile([P, OWN], f32, tag="psq", bufs=2)
                    for e in range(NE):
                        nc.tensor.matmul(psq, wstrip[:, e * P:(e + 1) * P],
                                         x1T[e], start=(e == 0),
                                         stop=(e == NE - 1))
                    nc.scalar.activation(out=qT[m], in_=psq, func=Ident,
                                         bias=bq_sb[:, m:m + 1])

                # Gathered K/V -> absolute positions (slot p is compile-time)
                for p in range(4):
                    lo, hi = CH * p, CH * (7 - p)
                    for kv in range(KH):
                        nc.sync.dma_start(out=kT[kv][:, lo:lo + CH],
                                          in_=kvx_out[p, kv][:, 0:CH])
                        nc.sync.dma_start(out=kT[kv][:, hi:hi + CH],
                                          in_=kvx_out[p, kv][:, CH:OWN])
                    for t, abs_t in enumerate(own_abs_tiles(p)):
                        nc.sync.dma_start(out=vtok[abs_t],
                                          in_=kvx_out[p, KH + t])

            # ---------------- Phase C: attention -------------------------
            with (
                tc.tile_pool(name="pC", bufs=1) as pC,
                tc.tile_pool(name="psC", bufs=1, space="PSUM") as psC,
            ):
                mlo = [pC.tile([P, OWN], bf16, tag=f"mlo{i}", name=f"mlo{i}")
                       for i in range(8)]
                mhi = [pC.tile([P, CH], bf16, tag=f"mhi{i}", name=f"mhi{i}")
                       for i in range(8)]
                for i in range(8):
                    nc.sync.dma_start(out=mlo[i], in_=mask_lo[i])
                    nc.sync.dma_start(out=mhi[i], in_=mask_hi[i])

                for h in range(H):
                    kv = h // G
                    ps_o = psC.tile([P, OWN], f32, tag="ps_o", bufs=1)
                    acc = pC.tile([P, OWN], bf16, tag="acc", bufs=2)
                    for sk in range(NSK):
                        lo_active = sk < 8
                        ncols = OWN if lo_active else CH
                        c0 = 0 if lo_active else CH
                        ps_s = psC.tile([P, OWN], f32, tag="ps_s", bufs=2)
                        nc.tensor.matmul(ps_s[:, c0:c0 + ncols],
                                         kT[kv][:, sk * P:(sk + 1) * P],
                                         qT[h][:, c0:c0 + ncols],
                                         start=True, stop=True)
                        exr = pC.tile([P, OWN], bf16, tag="exr", bufs=3)
                        nc.scalar.activation(out=exr[:, c0:c0 + ncols],
                                             in_=ps_s[:, c0:c0 + ncols],
                                             func=Exp, scale=EXP_SCALE)
                        m_t = mlo[sk] if lo_active else mhi[sk - 8]
                        nc.vector.tensor_tensor(
                            out=exr[:, c0:c0 + ncols],
                            in0=exr[:, c0:c0 + ncols], in1=m_t, op=mult)
                        # denominator accumulation (bf16, vector)
                        if sk == 0:
                            nc.vector.tensor_copy(acc, exr)
                        else:
                            nc.vector.tensor_tensor(
                                out=acc[:, c0:c0 + ncols],
                                in0=acc[:, c0:c0 + ncols],
                                in1=exr[:, c0:c0 + ncols], op=add)
                        # A/V accumulation: cols 0:256 chain ends at sk=7,
                        # cols 256:512 chain ends at sk=15
                        vt = vtok[sk][:, kv * P:(kv + 1) * P]
                        if sk < 7:
                            nc.tensor.matmul(ps_o, vt, exr,
                                             start=(sk == 0), stop=False)
                        elif sk == 7:
                            nc.tensor.matmul(ps_o[:, 0:CH], vt, exr[:, 0:CH],
                                             start=False, stop=True)
                            nc.tensor.matmul(ps_o[:, CH:OWN], vt,
                                             exr[:, CH:OWN],
                                             start=False, stop=False)
                        else:
                            nc.tensor.matmul(ps_o[:, CH:OWN], vt,
                                             exr[:, CH:OWN], start=False,
                                             stop=(sk == NSK - 1))
                    ps_den = psC.tile([1, OWN], f32, tag="ps_den", bufs=1)
                    nc.tensor.matmul(ps_den, ones_col, acc, start=True,
                                     stop=True)
                    rden = pC.tile([1, OWN], f32r, tag="rden", bufs=2)
                    with nc.allow_low_precision(reason="softmax denominator"):
                        nc.vector.reciprocal(out=rden, in_=ps_den)
                    ps_bc = psC.tile([P, OWN], f32, tag="ps_bc", bufs=1)
                    nc.tensor.matmul(ps_bc, ones_row.bitcast(f32r), rden,
                                     start=True, stop=True)
                    bc = pC.tile([P, OWN], f32, tag="bc", bufs=2)
                    nc.vector.tensor_copy(bc, ps_bc)
                    nc.vector.tensor_tensor(out=oT[h], in0=ps_o, in1=bc,
                                            op=mult)

            # ---------------- Phase D: o-proj + residual -----------------
            # matmul PSUM outputs are limited to one bank (512 fp32), so
            # process two 512-col chunks per pass (4 ms x 2 ec = 8 banks).
            with (
                tc.tile_pool(name="pD", bufs=1) as pD,
                tc.tile_pool(name="psD", bufs=1, space="PSUM") as psD,
            ):
                for half in range(2):
                    cols = slice(half * 1024, (half + 1) * 1024)
                    po = [[psD.tile([P, 512], f32, tag=f"po{ms}_{ec}", bufs=1,
                                    name=f"po{ms}_{ec}") for ec in range(2)]
                          for ms in range(NMS)]
                    for k in range(H):
                        wo_sb = pD.tile([P, 1024], bf16, tag="wo", bufs=4)
                        nc.sync.dma_start(out=wo_sb, in_=wo_b[k][:, cols])
                        for ms in range(NMS):
                            for ec in range(2):
                                nc.tensor.matmul(
                                    po[ms][ec], oT[k][:, ms * P:(ms + 1) * P],
                                    wo_sb[:, ec * 512:(ec + 1) * 512],
                                    start=(k == 0), stop=(k == H - 1))
                    for ms in range(NMS):
                        for ec in range(2):
                            c = slice(half * 1024 + ec * 512,
                                      half * 1024 + (ec + 1) * 512)
                            nc.vector.tensor_tensor(
                                out=xt[ms][:, c], in0=po[ms][ec],
                                in1=xt[ms][:, c], op=add)
                            nc.vector.tensor_tensor(
                                out=xt[ms][:, c], in0=xt[ms][:, c],
                                in1=bo_b[:, c], op=add)

        # ---------------- Phase E-G: LN2, MLP ---------------------------
        with tc.tile_pool(name="mkeep", bufs=1) as mkeep:
            x2T = [mkeep.tile([P, OWN], bf16, tag=f"x2T{e}", name=f"x2T{e}")
                   for e in range(NE)]
            hT = [mkeep.tile([P, OWN], bf16, tag=f"hT{i}", name=f"hT{i}")
                  for i in range(NF)]

            with (
                tc.tile_pool(name="pE", bufs=1) as pE,
                tc.tile_pool(name="psE", bufs=1, space="PSUM") as psE,
            ):
                _ln_transpose_strips(nc, pE, psE, xt, g2_sb, b2_sb, eps_t,
                                     ident, x2T, "E")

            with (
                tc.tile_pool(name="pF", bufs=1) as pF,
                tc.tile_pool(name="psF", bufs=1, space="PSUM") as psF,
            ):
                bu_sb = pF.tile([P, NF], f32)
                nc.sync.dma_start(out=bu_sb, in_=bu_c)
                for f in range(NF):
                    wstrip = pF.tile([P, NE * P], bf16, tag="wu", bufs=3)
                    nc.sync.dma_start(out=wstrip, in_=wu_b[f])
                    psh = psF.tile([P, OWN], f32, tag="psh", bufs=2)
                    for e in range(NE):
                        nc.tensor.matmul(psh, wstrip[:, e * P:(e + 1) * P],
                                         x2T[e], start=(e == 0),
                                         stop=(e == NE - 1))
                    nc.scalar.activation(out=hT[f], in_=psh, func=Gelu,
                                         bias=bu_sb[:, f:f + 1])

            with (
                tc.tile_pool(name="pG", bufs=1) as pG,
                tc.tile_pool(name="psG", bufs=1, space="PSUM") as psG,
            ):
                for half in range(2):
                    psd = [[psG.tile([P, 512], f32, tag=f"psd{i}_{ec}", bufs=1,
                                     name=f"psd{i}_{ec}") for ec in range(4)]
                           for i in range(2)]
                    for f in range(NF):
                        wrow = pG.tile([P, E], bf16, tag="wd", bufs=3)
                        nc.sync.dma_start(out=wrow, in_=wd_b[f])
                        for i in range(2):
                            ms = half * 2 + i
                            for ec in range(4):
                                nc.tensor.matmul(
                                    psd[i][ec], hT[f][:, ms * P:(ms + 1) * P],
                                    wrow[:, ec * 512:(ec + 1) * 512],
                                    start=(f == 0), stop=(f == NF - 1))
                    for i in range(2):
                        ms = half * 2 + i
                        ot = pG.tile([P, E], f32, tag="ot", bufs=2)
                        for ec in range(4):
                            c = slice(ec * 512, (ec + 1) * 512)
                            nc.vector.tensor_tensor(out=ot[:, c],
                                                    in0=psd[i][ec],
                                                    in1=xt[ms][:, c], op=add)
                        nc.vector.tensor_tensor(out=ot, in0=ot, in1=bd_b,
                                                op=add)
                        nc.sync.dma_start(out=out[ms * P:(ms + 1) * P, :],
                                          in_=ot)


_NC_CACHE = None
LAST_RESULTS = None


def _get_nc(debug=False):
    global _NC_CACHE
    if _NC_CACHE is None:
        nc = build(debug=debug)
        split_waits(nc)
        _NC_CACHE = nc
    return _NC_CACHE


def _prep_shared(wq, wk, wv, wo, wu, wd, bq, bk, bv, bo, bu, bd,
                 ln1_g, ln1_b, ln2_g, ln2_b):
    def strips(w, n):  # [E, n*128] -> [n, 128, E] with [f, p, (t d)] layout
        w = np.asarray(w, np.float32)
        return np.ascontiguousarray(
            w.reshape(NE, P, n, P).transpose(2, 1, 0, 3).reshape(n, P, E)
        ).astype(np_bf16)

    def rows(w, nr):  # [nr*128, E] -> [nr, 128, E]
        return np.ascontiguousarray(
            np.asarray(w, np.float32).reshape(nr, P, E)).astype(np_bf16)

    def cols(v, n):  # [n*128] -> [128, n]
        return np.ascontiguousarray(
            np.asarray(v, np.float32).reshape(n, P).T)

    return {
        "wq_b": strips(wq, H), "wk_b": strips(wk, KH), "wv_b": strips(wv, KH),
        "wo_b": rows(wo, H), "wu_b": strips(wu, NF), "wd_b": rows(wd, NF),
        "bq_c": cols(bq, H), "bk_c": cols(bk, KH), "bv_c": cols(bv, KH),
        "bu_c": cols(bu, NF),
        "g1_c": cols(ln1_g, NE), "b1_c": cols(ln1_b, NE),
        "g2_c": cols(ln2_g, NE), "b2_c": cols(ln2_b, NE),
        "bo_r": np.asarray(bo, np.float32), "bd_r": np.asarray(bd, np.float32),
    }


def _own_idx(j):
    return np.concatenate([np.arange(CH * j, CH * (j + 1)),
                           np.arange(CH * (7 - j), CH * (8 - j))])


def _masks(j):
    """Causal masks for core-slot j. q cols 0:256 = abs chunk j,
    cols 256:512 = abs chunk 7-j. mask_lo: sk tiles 0..7 (both halves);
    mask_hi: sk tiles 8..15 (high half only)."""
    q_lo = CH * j + np.arange(CH)[None, :]
    q_hi = CH * (7 - j) + np.arange(CH)[None, :]
    mask_lo = np.empty((8, P, OWN), np.float32)
    mask_hi = np.empty((8, P, CH), np.float32)
    for sk in range(8):
        k_idx = sk * P + np.arange(P)[:, None]
        mask_lo[sk, :, :CH] = (k_idx <= q_lo)
        mask_lo[sk, :, CH:] = (k_idx <= q_hi)
    for sk in range(8, 16):
        k_idx = sk * P + np.arange(P)[:, None]
        mask_hi[sk - 8] = (k_idx <= q_hi)
    return mask_lo.astype(np_bf16), mask_hi.astype(np_bf16)


def kernel(x, ln1_g, ln1_b, wq, bq, wk, bk, wv, bv, wo, bo, ln2_g, ln2_b,
           wu, bu, wd, bd):
    x = np.asarray(x, np.float32)
    shared = _prep_shared(wq, wk, wv, wo, wu, wd, bq, bk, bv, bo, bu, bd,
                          ln1_g, ln1_b, ln2_g, ln2_b)
    in_maps = []
    for core in range(8):
        b, j = divmod(core, 4)
        m = dict(shared)
        m["xow"] = np.ascontiguousarray(x[b, _own_idx(j)])
        m["mask_lo"], m["mask_hi"] = _masks(j)
        in_maps.append(m)

    nc = _get_nc()
    trace = bool(os.environ.get("KERNEL_TRACE"))
    res = bass_utils.run_bass_kernel_spmd(
        nc, in_maps, core_ids=list(range(8)), trace=trace)
    global LAST_RESULTS
    LAST_RESULTS = res
    out = np.empty((B, S, E), np.float32)
    for core in range(8):
        b, j = divmod(core, 4)
        out[b, _own_idx(j)] = res.results[core]["out"]
    return out


# revision 14
# speedup vs baseline: 1.5200x; 1.0296x over previous
"""GPT-2 transformer block on 8 trn2 NeuronCores (Bass/Tile).

Sharding: balanced-causal token split. Core c = 4*b + j handles batch b and
owns two 256-token chunks: abs chunks j and 7-j (512 tokens total), so every
core has the same causal attention workload. Each core LayerNorms + QKV-
projects only its own tokens; K/V are exchanged within the 4-core batch group
via an HBM AllGather, with each contribution landing at absolute positions
keyed by the compile-time replica slot. Attention uses uniform loop bounds
(sk tiles 0..7 for the low q-half, 0..15 for the high q-half) with per-core
causal masks supplied as input data; per-core structural skip is impossible
under SPMD. MLP is token-parallel (no collectives).

All matmul operands are bf16 (fp32 PSUM accumulation); the residual stream is
fp32 and never leaves SBUF between attention and MLP. Weight tensors are
pre-transposed on the host so every weight DMA is a contiguous [128, 2048]
block (4KB per partition line). Softmax: exp on the scalar engine (scores
bounded, no max subtraction), causal mask multiply + denominator accumulation
on the vector engine, single ones-column matmul per head for the partition
reduction.
"""
import math
import os
import sys
import types

sys.path.insert(0, '/opt/trn_rl_repo')

import numpy as np
from ml_dtypes import bfloat16 as np_bf16


def _install_ntff_shim():
    """concourse's trace path imports antenv.axon_hooks, which this image
    lacks; give it a functional stand-in so trace=True doesn't crash."""
    try:
        import antenv.axon_hooks  # noqa: F401
        return
    except ImportError:
        pass
    try:
        import antenv
    except ImportError:
        return
    mod = types.ModuleType("antenv.axon_hooks")
    mod._hook = None

    def set_axon_ntff_profile_hook(h):
        mod._hook = h

    def get_axon_ntff_profile_hook():
        return mod._hook

    mod.set_axon_ntff_profile_hook = set_axon_ntff_profile_hook
    mod.get_axon_ntff_profile_hook = get_axon_ntff_profile_hook
    sys.modules["antenv.axon_hooks"] = mod
    antenv.axon_hooks = mod
    try:
        from trn_agent_boot.trn_boot import _ntff_profile_via_ctypes
        hook = _ntff_profile_via_ctypes('/opt/axon/libaxon_pjrt.so')
        if hook is not None:
            set_axon_ntff_profile_hook(hook)
    except Exception:
        pass


_install_ntff_shim()

import concourse.bass as bass
import concourse.tile as tile
from concourse import mybir, bass_utils
from concourse.masks import make_identity

P = 128
B, S, E = 2, 2048, 2048
H, D, KH, G = 16, 128, 4, 4
F = 8192
OWN = 512                 # tokens owned per core (two 256 chunks)
CH = 256                  # ownership chunk
NE = E // P               # 16
NSK = S // P              # 16
NF = F // P               # 64
NMS = OWN // P            # 4
f32 = mybir.dt.float32
f32r = mybir.dt.float32r
bf16 = mybir.dt.bfloat16
EXP_SCALE = 1.0 / math.sqrt(D)


def split_waits(nc, maxw=1):
    """This walrus build supports at most one sync-wait per instruction;
    hoist excess waits onto same-engine NoOps placed before the owner."""
    n = 0
    for fn in nc.m.functions:
        for blk in fn.blocks:
            new_insts = []
            for inst in blk.instructions:
                si = inst.sync_info
                if si is not None and si.on_wait and len(si.on_wait) > maxw:
                    waits = list(si.on_wait)
                    excess, keep = waits[:-maxw], waits[-maxw:]
                    for ci, w in enumerate(excess):
                        new_insts.append(mybir.InstNoOp(
                            name=f"{inst.name}-ws{ci}", engine=inst.engine,
                            sync_info=mybir.SyncInfo(on_wait=[w], on_update=[])))
                        n += 1
                    inst.sync_info = mybir.SyncInfo(
                        on_wait=keep, on_update=list(si.on_update or []))
                new_insts.append(inst)
            blk.instructions = new_insts
    return n


def own_abs_tiles(p):
    """Absolute 128-token sk-tile indices of core-slot p's four token tiles."""
    return [2 * p, 2 * p + 1, 14 - 2 * p, 15 - 2 * p]


def _ln_normalize(nc, pool, x_tile, eps_t, out_tile, tag):
    """out = (x - mean) * rsqrt(var + eps) along the free dim (E), fp32."""
    stats = pool.tile([P, E // 512, 6], f32, tag=f"{tag}_stats")
    for i in range(E // 512):
        nc.vector.bn_stats(out=stats[:, i, :], in_=x_tile[:, i * 512:(i + 1) * 512])
    mv = pool.tile([P, 2], f32, tag=f"{tag}_mv")
    nc.vector.bn_aggr(out=mv, in_=stats)
    rstd = pool.tile([P, 1], f32, tag=f"{tag}_rstd")
    nc.scalar.activation(out=rstd, in_=mv[:, 1:2],
                         func=mybir.ActivationFunctionType.Sqrt, bias=eps_t)
    nc.vector.reciprocal(out=rstd, in_=rstd)
    nc.vector.tensor_scalar(out=out_tile, in0=x_tile, scalar1=mv[:, 0:1],
                            scalar2=rstd, op0=mybir.AluOpType.subtract,
                            op1=mybir.AluOpType.mult)


def _ln_transpose_strips(nc, pool, psum_pool, x_tiles, g_col, b_col, eps_t,
                         ident, strips, tag):
    """LayerNorm token tiles and transpose into e-major bf16 strips with the
    affine (g, b) fused into the post-transpose copy (g/b become per-partition
    scalars after the transpose)."""
    Ident = mybir.ActivationFunctionType.Identity
    for t, x_t in enumerate(x_tiles):
        xn = pool.tile([P, E], f32, tag=f"{tag}_xn", bufs=2)
        _ln_normalize(nc, pool, x_t, eps_t, xn, tag)
        for e in range(NE):
            tp = psum_pool.tile([P, P], f32, tag=f"{tag}_tp", bufs=2)
            nc.tensor.transpose(tp, xn[:, e * P:(e + 1) * P], ident)
            nc.scalar.activation(out=strips[e][:, t * P:(t + 1) * P], in_=tp,
                                 func=Ident, scale=g_col[:, e:e + 1],
                                 bias=b_col[:, e:e + 1])


def build(debug=False):
    nc = bass.Bass("TRN2", target_bir_lowering=False, debug=debug,
                   num_devices=8)

    xow = nc.dram_tensor("xow", [OWN, E], f32, kind="ExternalInput").ap()
    mask_lo = nc.dram_tensor("mask_lo", [8, P, OWN], bf16,
                             kind="ExternalInput").ap()
    mask_hi = nc.dram_tensor("mask_hi", [8, P, CH], bf16,
                             kind="ExternalInput").ap()
    wq_b = nc.dram_tensor("wq_b", [H, P, NE * P], bf16, kind="ExternalInput").ap()
    wk_b = nc.dram_tensor("wk_b", [KH, P, NE * P], bf16, kind="ExternalInput").ap()
    wv_b = nc.dram_tensor("wv_b", [KH, P, NE * P], bf16, kind="ExternalInput").ap()
    wo_b = nc.dram_tensor("wo_b", [H, P, E], bf16, kind="ExternalInput").ap()
    wu_b = nc.dram_tensor("wu_b", [NF, P, NE * P], bf16, kind="ExternalInput").ap()
    wd_b = nc.dram_tensor("wd_b", [NF, P, E], bf16, kind="ExternalInput").ap()
    bq_c = nc.dram_tensor("bq_c", [P, H], f32, kind="ExternalInput").ap()
    bk_c = nc.dram_tensor("bk_c", [P, KH], f32, kind="ExternalInput").ap()
    bv_c = nc.dram_tensor("bv_c", [P, KH], f32, kind="ExternalInput").ap()
    bu_c = nc.dram_tensor("bu_c", [P, NF], f32, kind="ExternalInput").ap()
    g1_c = nc.dram_tensor("g1_c", [P, NE], f32, kind="ExternalInput").ap()
    b1_c = nc.dram_tensor("b1_c", [P, NE], f32, kind="ExternalInput").ap()
    g2_c = nc.dram_tensor("g2_c", [P, NE], f32, kind="ExternalInput").ap()
    b2_c = nc.dram_tensor("b2_c", [P, NE], f32, kind="ExternalInput").ap()
    bo_r = nc.dram_tensor("bo_r", [E], f32, kind="ExternalInput").ap()
    bd_r = nc.dram_tensor("bd_r", [E], f32, kind="ExternalInput").ap()
    out = nc.dram_tensor("out", [OWN, E], f32, kind="ExternalOutput").ap()

    with tile.TileContext(nc) as tc:
        _build_body(nc, tc, locals())
    return nc


def _build_body(nc, tc, t_):
    xow, mask_lo, mask_hi = t_["xow"], t_["mask_lo"], t_["mask_hi"]
    wq_b, wk_b, wv_b, wo_b, wu_b, wd_b = (t_[k] for k in
                                          ("wq_b", "wk_b", "wv_b", "wo_b",
                                           "wu_b", "wd_b"))
    bq_c, bk_c, bv_c, bu_c = (t_[k] for k in ("bq_c", "bk_c", "bv_c", "bu_c"))
    g1_c, b1_c, g2_c, b2_c = (t_[k] for k in ("g1_c", "b1_c", "g2_c", "b2_c"))
    bo_r, bd_r, out = t_["bo_r"], t_["bd_r"], t_["out"]
    Ident = mybir.ActivationFunctionType.Identity
    Exp = mybir.ActivationFunctionType.Exp
    Gelu = mybir.ActivationFunctionType.Gelu
    mult = mybir.AluOpType.mult
    add = mybir.AluOpType.add

    with (
        tc.tile_pool(name="persist", bufs=1) as persist,
        tc.tile_pool(name="dram", bufs=1, space="DRAM") as dram,
    ):
        ident = persist.tile([P, P], f32)
        make_identity(nc, ident)
        eps_t = persist.tile([P, 1], f32)
        nc.vector.memset(eps_t, 1e-5)
        ones_col = persist.tile([P, 1], bf16)   # lhsT for den (K=P, M=1)
        nc.vector.memset(ones_col, 1.0)
        ones_row = persist.tile([1, P], f32)    # lhsT for broadcast (K=1, M=P)
        nc.vector.memset(ones_row, 1.0)
        bq_sb = persist.tile([P, H], f32)
        nc.sync.dma_start(out=bq_sb, in_=bq_c)
        bk_sb = persist.tile([P, KH], f32)
        nc.sync.dma_start(out=bk_sb, in_=bk_c)
        bv_sb = persist.tile([P, KH], f32)
        nc.sync.dma_start(out=bv_sb, in_=bv_c)
        g1_sb = persist.tile([P, NE], f32)
        nc.sync.dma_start(out=g1_sb, in_=g1_c)
        b1_sb = persist.tile([P, NE], f32)
        nc.sync.dma_start(out=b1_sb, in_=b1_c)
        g2_sb = persist.tile([P, NE], f32)
        nc.sync.dma_start(out=g2_sb, in_=g2_c)
        b2_sb = persist.tile([P, NE], f32)
        nc.sync.dma_start(out=b2_sb, in_=b2_c)
        bo_b = persist.tile([P, E], f32)
        nc.sync.dma_start(out=bo_b, in_=bo_r.unsqueeze(0).to_broadcast((P, E)))
        bd_b = persist.tile([P, E], f32)
        nc.sync.dma_start(out=bd_b, in_=bd_r.unsqueeze(0).to_broadcast((P, E)))

        # DRAM bounce buffers for the K/V AllGather.
        # units 0..KH-1: kT (feat-major); units KH..KH+3: vtok (tok-major)
        kvx_in = dram.tile([KH + NMS, P, OWN], bf16)
        kvx_out = dram.tile([4, KH + NMS, P, OWN], bf16)

        # xt holds the residual stream: input x during attention, then is
        # overwritten in place with xmid = x + attn_out for the MLP phases.
        xt = [persist.tile([P, E], f32, name=f"xt{t}") for t in range(NMS)]

        with tc.tile_pool(name="attn_keep", bufs=1) as akeep:
            qT = [akeep.tile([P, OWN], bf16, tag=f"qT{i}", name=f"qT{i}")
                  for i in range(H)]
            kT = [akeep.tile([P, S], bf16, tag=f"kT{i}", name=f"kT{i}")
                  for i in range(KH)]
            vtok = [akeep.tile([P, KH * P], bf16, tag=f"vt{i}", name=f"vt{i}")
                    for i in range(NSK)]
            oT = [akeep.tile([P, OWN], bf16, tag=f"oT{i}", name=f"oT{i}")
                  for i in range(H)]

            # ---------------- Phase A: LN1 + QKV for own tokens ----------
            with (
                tc.tile_pool(name="pA", bufs=1) as pA,
                tc.tile_pool(name="psA", bufs=1, space="PSUM") as psA,
            ):
                for t in range(NMS):
                    nc.sync.dma_start(out=xt[t], in_=xow[t * P:(t + 1) * P, :])
                x1T = [pA.tile([P, OWN], bf16, tag=f"x1T{e}", name=f"x1T{e}")
                       for e in range(NE)]
                _ln_transpose_strips(nc, pA, psA, xt, g1_sb, b1_sb, eps_t,
                                     ident, x1T, "A")

                # K/V projections for own tokens -> exchange staging
                kstage = [pA.tile([P, OWN], bf16, tag=f"ks{kv}", name=f"ks{kv}")
                          for kv in range(KH)]
                vstage = [pA.tile([P, KH * P], bf16, tag=f"vs{t}", name=f"vs{t}")
                          for t in range(NMS)]
                for kv_or_v in range(2):
                    w_src, b_sb = ((wk_b, bk_sb), (wv_b, bv_sb))[kv_or_v]
                    for m in range(KH):
                        wstrip = pA.tile([P, NE * P], bf16, tag="wkv", bufs=3)
                        nc.sync.dma_start(out=wstrip, in_=w_src[m])
                        pskv = psA.tile([P, OWN], f32, tag="pskv", bufs=2)
                        for e in range(NE):
                            nc.tensor.matmul(pskv, wstrip[:, e * P:(e + 1) * P],
                                             x1T[e], start=(e == 0),
                                             stop=(e == NE - 1))
                        if kv_or_v == 0:
                            nc.scalar.activation(out=kstage[m], in_=pskv,
                                                 func=Ident,
                                                 bias=b_sb[:, m:m + 1])
                        else:
                            vf = pA.tile([P, OWN], f32, tag="vf", bufs=2)
                            nc.scalar.activation(out=vf, in_=pskv, func=Ident,
                                                 bias=b_sb[:, m:m + 1])
                            for t in range(NMS):
                                tp = psA.tile([P, P], f32, tag="vtp", bufs=2)
                                nc.tensor.transpose(
                                    tp, vf[:, t * P:(t + 1) * P], ident)
                                nc.scalar.copy(
                                    vstage[t][:, m * P:(m + 1) * P], tp)
                for kv in range(KH):
                    nc.gpsimd.dma_start(kvx_in[kv], kstage[kv])
                for t in range(NMS):
                    nc.gpsimd.dma_start(kvx_in[KH + t], vstage[t])
                nc.gpsimd.collective_compute(
                    "AllGather", mybir.AluOpType.bypass,
                    replica_groups=[[0, 1, 2, 3], [4, 5, 6, 7]],
                    ins=[kvx_in.opt()], outs=[kvx_out.opt()])

                # Q projections (overlap the collective)
                for m in range(H):
                    wstrip = pA.tile([P, NE * P], bf16, tag="wq", bufs=3)
                    nc.sync.dma_start(out=wstrip, in_=wq_b[m])
                    psq = psA.tile([P, OWN], f32, tag="psq", bufs=2)
                    for e in range(NE):
                        nc.tensor.matmul(psq, wstrip[:, e * P:(e + 1) * P],
                                         x1T[e], start=(e == 0),
                                         stop=(e == NE - 1))
                    nc.scalar.activation(out=qT[m], in_=psq, func=Ident,
                                         bias=bq_sb[:, m:m + 1])

                # Gathered K/V -> absolute positions (slot p is compile-time)
                for p in range(4):
                    lo, hi = CH * p, CH * (7 - p)
                    for kv in range(KH):
                        nc.sync.dma_start(out=kT[kv][:, lo:lo + CH],
                                          in_=kvx_out[p, kv][:, 0:CH])
                        nc.sync.dma_start(out=kT[kv][:, hi:hi + CH],
                                          in_=kvx_out[p, kv][:, CH:OWN])
                    for t, abs_t in enumerate(own_abs_tiles(p)):
                        nc.sync.dma_start(out=vtok[abs_t],
                                          in_=kvx_out[p, KH + t])

            # ---------------- Phase C: attention -------------------------
            with (
                tc.tile_pool(name="pC", bufs=1) as pC,
                tc.tile_pool(name="psC", bufs=1, space="PSUM") as psC,
            ):
                mlo = [pC.tile([P, OWN], bf16, tag=f"mlo{i}", name=f"mlo{i}")
                       for i in range(8)]
                mhi = [pC.tile([P, CH], bf16, tag=f"mhi{i}", name=f"mhi{i}")
                       for i in range(8)]
                for i in range(8):
                    nc.sync.dma_start(out=mlo[i], in_=mask_lo[i])
                    nc.sync.dma_start(out=mhi[i], in_=mask_hi[i])

                for h in range(H):
                    kv = h // G
                    ps_o = psC.tile([P, OWN], f32, tag="ps_o", bufs=1)
                    ps_den = psC.tile([1, OWN], f32, tag="ps_den", bufs=1)
                    for sk in range(NSK):
                        lo_active = sk < 8
                        ncols = OWN if lo_active else CH
                        c0 = 0 if lo_active else CH
                        ps_s = psC.tile([P, OWN], f32, tag="ps_s", bufs=2)
                        nc.tensor.matmul(ps_s[:, c0:c0 + ncols],
                                         kT[kv][:, sk * P:(sk + 1) * P],
                                         qT[h][:, c0:c0 + ncols],
                                         start=True, stop=True)
                        exr = pC.tile([P, OWN], bf16, tag="exr", bufs=3)
                        nc.scalar.activation(out=exr[:, c0:c0 + ncols],
                                             in_=ps_s[:, c0:c0 + ncols],
                                             func=Exp, scale=EXP_SCALE)
                        m_t = mlo[sk] if lo_active else mhi[sk - 8]
                        nc.vector.tensor_tensor(
                            out=exr[:, c0:c0 + ncols],
                            in0=exr[:, c0:c0 + ncols], in1=m_t, op=mult)
                        # A/V + denominator accumulation on PE: cols 0:256
                        # chain ends at sk=7, cols 256:512 chain ends at sk=15
                        vt = vtok[sk][:, kv * P:(kv + 1) * P]
                        if sk < 7:
                            nc.tensor.matmul(ps_o, vt, exr,
                                             start=(sk == 0), stop=False)
                            nc.tensor.matmul(ps_den, ones_col, exr,
                                             start=(sk == 0), stop=False)
                        elif sk == 7:
                            nc.tensor.matmul(ps_o[:, 0:CH], vt, exr[:, 0:CH],
                                             start=False, stop=True)
                            nc.tensor.matmul(ps_o[:, CH:OWN], vt,
                                             exr[:, CH:OWN],
                                             start=False, stop=False)
                            nc.tensor.matmul(ps_den[:, 0:CH], ones_col,
                                             exr[:, 0:CH], start=False,
                                             stop=True)
                            nc.tensor.matmul(ps_den[:, CH:OWN], ones_col,
                                             exr[:, CH:OWN], start=False,
                                             stop=False)
                        else:
                            nc.tensor.matmul(ps_o[:, CH:OWN], vt,
                                             exr[:, CH:OWN], start=False,
                                             stop=(sk == NSK - 1))
                            nc.tensor.matmul(ps_den[:, CH:OWN], ones_col,
                                             exr[:, CH:OWN], start=False,
                                             stop=(sk == NSK - 1))
                    rden = pC.tile([1, OWN], f32r, tag="rden", bufs=2)
                    with nc.allow_low_precision(reason="softmax denominator"):
                        nc.vector.reciprocal(out=rden, in_=ps_den)
                    ps_bc = psC.tile([P, OWN], f32, tag="ps_bc", bufs=1)
                    nc.tensor.matmul(ps_bc, ones_row.bitcast(f32r), rden,
                                     start=True, stop=True)
                    bc = pC.tile([P, OWN], f32, tag="bc", bufs=2)
                    nc.vector.tensor_copy(bc, ps_bc)
                    nc.vector.tensor_tensor(out=oT[h], in0=ps_o, in1=bc,
                                            op=mult)

            # ---------------- Phase D: o-proj + residual -----------------
            # matmul PSUM outputs are limited to one bank (512 fp32), so
            # process two 512-col chunks per pass (4 ms x 2 ec = 8 banks).
            with (
                tc.tile_pool(name="pD", bufs=1) as pD,
                tc.tile_pool(name="psD", bufs=1, space="PSUM") as psD,
            ):
                for half in range(2):
                    cols = slice(half * 1024, (half + 1) * 1024)
                    po = [[psD.tile([P, 512], f32, tag=f"po{ms}_{ec}", bufs=1,
                                    name=f"po{ms}_{ec}") for ec in range(2)]
                          for ms in range(NMS)]
                    for k in range(H):
                        wo_sb = pD.tile([P, 1024], bf16, tag="wo", bufs=4)
                        nc.sync.dma_start(out=wo_sb, in_=wo_b[k][:, cols])
                        for ms in range(NMS):
                            for ec in range(2):
                                nc.tensor.matmul(
                                    po[ms][ec], oT[k][:, ms * P:(ms + 1) * P],
                                    wo_sb[:, ec * 512:(ec + 1) * 512],
                                    start=(k == 0), stop=(k == H - 1))
                    for ms in range(NMS):
                        for ec in range(2):
                            c = slice(half * 1024 + ec * 512,
                                      half * 1024 + (ec + 1) * 512)
                            nc.vector.tensor_tensor(
                                out=xt[ms][:, c], in0=po[ms][ec],
                                in1=xt[ms][:, c], op=add)
                            nc.vector.tensor_tensor(
                                out=xt[ms][:, c], in0=xt[ms][:, c],
                                in1=bo_b[:, c], op=add)

        # ---------------- Phase E-G: LN2, MLP ---------------------------
        with tc.tile_pool(name="mkeep", bufs=1) as mkeep:
            x2T = [mkeep.tile([P, OWN], bf16, tag=f"x2T{e}", name=f"x2T{e}")
                   for e in range(NE)]
            hT = [mkeep.tile([P, OWN], bf16, tag=f"hT{i}", name=f"hT{i}")
                  for i in range(NF)]

            with (
                tc.tile_pool(name="pE", bufs=1) as pE,
                tc.tile_pool(name="psE", bufs=1, space="PSUM") as psE,
            ):
                _ln_transpose_strips(nc, pE, psE, xt, g2_sb, b2_sb, eps_t,
                                     ident, x2T, "E")

            with (
                tc.tile_pool(name="pF", bufs=1) as pF,
                tc.tile_pool(name="psF", bufs=1, space="PSUM") as psF,
            ):
                bu_sb = pF.tile([P, NF], f32)
                nc.sync.dma_start(out=bu_sb, in_=bu_c)
                for f in range(NF):
                    wstrip = pF.tile([P, NE * P], bf16, tag="wu", bufs=3)
                    nc.sync.dma_start(out=wstrip, in_=wu_b[f])
                    psh = psF.tile([P, OWN], f32, tag="psh", bufs=2)
                    for e in range(NE):
                        nc.tensor.matmul(psh, wstrip[:, e * P:(e + 1) * P],
                                         x2T[e], start=(e == 0),
                                         stop=(e == NE - 1))
                    nc.scalar.activation(out=hT[f], in_=psh, func=Gelu,
                                         bias=bu_sb[:, f:f + 1])

            with (
                tc.tile_pool(name="pG", bufs=1) as pG,
                tc.tile_pool(name="psG", bufs=1, space="PSUM") as psG,
            ):
                # pass over ec-halves so wd is read exactly once
                # (4 ms x 2 ec-chunks = 8 psum banks per pass)
                ot = [pG.tile([P, E], f32, tag=f"og{ms}", name=f"og{ms}")
                      for ms in range(NMS)]
                for half in range(2):
                    cols = slice(half * 1024, (half + 1) * 1024)
                    psd = [[psG.tile([P, 512], f32, tag=f"psd{ms}_{ec}",
                                     bufs=1, name=f"psd{ms}_{ec}")
                            for ec in range(2)] for ms in range(NMS)]
                    for f in range(NF):
                        wrow = pG.tile([P, 1024], bf16, tag="wd", bufs=3)
                        nc.sync.dma_start(out=wrow, in_=wd_b[f][:, cols])
                        for ms in range(NMS):
                            for ec in range(2):
                                nc.tensor.matmul(
                                    psd[ms][ec], hT[f][:, ms * P:(ms + 1) * P],
                                    wrow[:, ec * 512:(ec + 1) * 512],
                                    start=(f == 0), stop=(f == NF - 1))
                    for ms in range(NMS):
                        for ec in range(2):
                            c = slice(half * 1024 + ec * 512,
                                      half * 1024 + (ec + 1) * 512)
                            nc.vector.tensor_tensor(out=ot[ms][:, c],
                                                    in0=psd[ms][ec],
                                                    in1=xt[ms][:, c], op=add)
                            nc.vector.tensor_tensor(out=ot[ms][:, c],
                                                    in0=ot[ms][:, c],
                                                    in1=bd_b[:, c], op=add)
                for ms in range(NMS):
                    nc.sync.dma_start(out=out[ms * P:(ms + 1) * P, :],
                                      in_=ot[ms])


_NC_CACHE = None
LAST_RESULTS = None


def _get_nc(debug=False):
    global _NC_CACHE
    if _NC_CACHE is None:
        nc = build(debug=debug)
        split_waits(nc)
        _NC_CACHE = nc
    return _NC_CACHE


def _prep_shared(wq, wk, wv, wo, wu, wd, bq, bk, bv, bo, bu, bd,
                 ln1_g, ln1_b, ln2_g, ln2_b):
    def strips(w, n):  # [E, n*128] -> [n, 128, E] with [f, p, (t d)] layout
        w = np.asarray(w, np.float32)
        return np.ascontiguousarray(
            w.reshape(NE, P, n, P).transpose(2, 1, 0, 3).reshape(n, P, E)
        ).astype(np_bf16)

    def rows(w, nr):  # [nr*128, E] -> [nr, 128, E]
        return np.ascontiguousarray(
            np.asarray(w, np.float32).reshape(nr, P, E)).astype(np_bf16)

    def cols(v, n):  # [n*128] -> [128, n]
        return np.ascontiguousarray(
            np.asarray(v, np.float32).reshape(n, P).T)

    return {
        "wq_b": strips(wq, H), "wk_b": strips(wk, KH), "wv_b": strips(wv, KH),
        "wo_b": rows(wo, H), "wu_b": strips(wu, NF), "wd_b": rows(wd, NF),
        "bq_c": cols(bq, H), "bk_c": cols(bk, KH), "bv_c": cols(bv, KH),
        "bu_c": cols(bu, NF),
        "g1_c": cols(ln1_g, NE), "b1_c": cols(ln1_b, NE),
        "g2_c": cols(ln2_g, NE), "b2_c": cols(ln2_b, NE),
        "bo_r": np.asarray(bo, np.float32), "bd_r": np.asarray(bd, np.float32),
    }


def _own_idx(j):
    return np.concatenate([np.arange(CH * j, CH * (j + 1)),
                           np.arange(CH * (7 - j), CH * (8 - j))])


def _masks(j):
    """Causal masks for core-slot j. q cols 0:256 = abs chunk j,
    cols 256:512 = abs chunk 7-j. mask_lo: sk tiles 0..7 (both halves);
    mask_hi: sk tiles 8..15 (high half only)."""
    q_lo = CH * j + np.arange(CH)[None, :]
    q_hi = CH * (7 - j) + np.arange(CH)[None, :]
    mask_lo = np.empty((8, P, OWN), np.float32)
    mask_hi = np.empty((8, P, CH), np.float32)
    for sk in range(8):
        k_idx = sk * P + np.arange(P)[:, None]
        mask_lo[sk, :, :CH] = (k_idx <= q_lo)
        mask_lo[sk, :, CH:] = (k_idx <= q_hi)
    for sk in range(8, 16):
        k_idx = sk * P + np.arange(P)[:, None]
        mask_hi[sk - 8] = (k_idx <= q_hi)
    return mask_lo.astype(np_bf16), mask_hi.astype(np_bf16)


def kernel(x, ln1_g, ln1_b, wq, bq, wk, bk, wv, bv, wo, bo, ln2_g, ln2_b,
           wu, bu, wd, bd):
    x = np.asarray(x, np.float32)
    shared = _prep_shared(wq, wk, wv, wo, wu, wd, bq, bk, bv, bo, bu, bd,
                          ln1_g, ln1_b, ln2_g, ln2_b)
    in_maps = []
    for core in range(8):
        b, j = divmod(core, 4)
        m = dict(shared)
        m["xow"] = np.ascontiguousarray(x[b, _own_idx(j)])
        m["mask_lo"], m["mask_hi"] = _masks(j)
        in_maps.append(m)

    nc = _get_nc()
    trace = bool(os.environ.get("KERNEL_TRACE"))
    res = bass_utils.run_bass_kernel_spmd(
        nc, in_maps, core_ids=list(range(8)), trace=trace)
    global LAST_RESULTS
    LAST_RESULTS = res
    out = np.empty((B, S, E), np.float32)
    for core in range(8):
        b, j = divmod(core, 4)
        out[b, _own_idx(j)] = res.results[core]["out"]
    return out
